# revision 36
# baseline (speedup 1.0000x reference)
"""Trainium2 Bass kernel for BatchTemporalContrastiveLoss.

Computation (see reference):
  project(x) = l2norm(layernorm(relu(x @ W1 + b1) @ W2 + b2) * g + beta)
  anchors    = project(hidden_states)     [B, 128]
  positives  = project(positive_hidden)   [B, 128]
  pool       = concat([anchors, positives])          [2B, 128]
  negs       = pool[neg_idx]                         [B, K, 128]
  loss       = mean(-pos_sim + logsumexp([pos_sim, neg_sims]))

Strategy: pure data-parallel over B across 8 NeuronCores.
- Host pre-transposes activations to bf16 feature-major shards; the MLP runs
  feature-major (no on-chip input transposes); one wide DMA-xbar transpose
  per 512-row block flips projections to row-major for the normalize stage.
- gamma==1/beta==0 fast path: LN + L2-normalize collapses to
  (p - mu) * rsqrt(128*var)  (the LN eps cancels exactly), halving the
  normalize DVE work.
- bf16 pool shards to DRAM; the anchors AllGather overlaps the positives
  projection; the positives AllGather is split into chunks so most of it
  hides under the projection as well.
- Negatives fetched with dma_gather in merged 4-tile calls (4096 indices
  per instruction) to amortize per-instruction drain serialization:
  indices >> 2 select 1KB super-rows (4 bf16 rows, fits int16); a
  host-prepared penalty mask (-8 on wrong sub-rows) kills bad candidates
  inside the final logsumexp.
- Similarities: DVE mul + direct tensor_reduce; one batched logsumexp at
  the end. Host sums 8 per-core [128, 2] accumulators.
"""

import os
import sys

import numpy as np

for _p in ("/opt/trn_rl_repo", "/root/.axon_site/_ro/trn_rl_repo"):
    if os.path.isdir(_p) and _p not in sys.path:
        sys.path.append(_p)

import ml_dtypes  # noqa: E402

import concourse.bacc as bacc  # noqa: E402
import concourse.bass as bass  # noqa: E402
import concourse.tile as tile  # noqa: E402
from concourse import mybir  # noqa: E402
from concourse.bass_utils import run_bass_kernel_spmd  # noqa: E402

F32 = mybir.dt.float32
BF16 = mybir.dt.bfloat16
FP8 = mybir.dt.float8e4
I16 = mybir.dt.int16
I32 = mybir.dt.int32
PM2 = mybir.MatmulPerfMode.DoubleRow
AF = mybir.ActivationFunctionType
OP = mybir.AluOpType

HID = 512
MID = 256
PROJ = 128
K = 8
TEMP = 0.07
EPS_LN = 1e-5
P = 128
NB = 512  # rows per phase-1 block

N_CORES = 8
B_FULL = 65536
BS = B_FULL // N_CORES  # rows per core per input (8192)

NCHUNK_A = 2  # anchors AllGather chunks
NCHUNK_P = 2  # positives AllGather chunks
GCH = 1  # tiles merged per dma_gather call (>2 overflows the SWDGE desc ring)


def build_loss_kernel(
    bs: int = BS,
    ncores: int = N_CORES,
    use_b1: bool = False,
    use_b2: bool = False,
    use_gamma: bool = False,
    use_beta: bool = False,
):
    nt = bs // P  # 128-row tiles per input
    nblk = bs // NB  # 512-row blocks per input
    assert bs % NB == 0
    pool_rows = 2 * bs * ncores
    affine = use_gamma or use_beta

    nsq = int(os.environ.get("BTCL_NSQ", "4"))
    gch = int(os.environ.get("BTCL_GCH", str(GCH)))
    gb = os.environ.get("BTCL_GATHER", "super")  # "exact" | "super"
    fp8mm = os.environ.get("BTCL_FP8MM", "0") == "1"
    mdt = FP8 if fp8mm else BF16  # matmul operand dtype
    nc = bacc.Bacc(trn_type="TRN2", num_devices=ncores, num_swdge_queues=nsq)

    # Per-core external inputs. xa/xp are TRANSPOSED shards [HID, bs].
    xa = nc.dram_tensor("xa", [HID, bs], mdt, kind="ExternalInput").ap()
    xp = nc.dram_tensor("xp", [HID, bs], mdt, kind="ExternalInput").ap()
    # w1 blocks: w1[p, kc, mc, j] = W1[kc*128+p, mc*128+j]
    w1 = nc.dram_tensor(
        "w1", [P, HID // P, MID // P, P], mdt, kind="ExternalInput"
    ).ap()
    # w2 blocks: w2[p, kc, j] = W2[kc*128+p, j]
    w2 = nc.dram_tensor("w2", [P, MID // P, PROJ], mdt, kind="ExternalInput").ap()
    if gb == "exact":
        nidx = nc.dram_tensor("nidx", [bs, K], I32, kind="ExternalInput").ap()
        gidx = pen = None
    else:
        # merged gather indices: [nt/gch, 128, gch*K*128/16]
        gidx = nc.dram_tensor(
            "gidx", [nt // gch, P, gch * K * P // 16], I16, kind="ExternalInput"
        ).ap()
        # penalties pre-transposed on host to [128, nt, 4K]
        pen = nc.dram_tensor("pen", [P, nt, 4 * K], BF16, kind="ExternalInput").ap()
        nidx = None
    ident = nc.dram_tensor("ident", [P, P], BF16, kind="ExternalInput").ap()
    b1 = (
        nc.dram_tensor("b1v", [P, MID // P], F32, kind="ExternalInput").ap()
        if use_b1
        else None
    )
    b2 = (
        nc.dram_tensor("b2v", [P, 1], F32, kind="ExternalInput").ap()
        if use_b2
        else None
    )
    gam = (
        nc.dram_tensor("gam", [1, PROJ], F32, kind="ExternalInput").ap()
        if use_gamma
        else None
    )
    bet = (
        nc.dram_tensor("bet", [1, PROJ], F32, kind="ExternalInput").ap()
        if use_beta
        else None
    )

    out_acc = nc.dram_tensor("out_acc", [P, 2], F32, kind="ExternalOutput").ap()

    acs = bs // NCHUNK_A  # anchor-shard chunk rows
    pool_ac = [
        nc.dram_tensor(f"pool_a{i}", [acs, PROJ], BF16, kind="Internal").ap()
        for i in range(NCHUNK_A)
    ]
    pcs = bs // NCHUNK_P  # positive-shard chunk rows
    pool_pc = [
        nc.dram_tensor(f"pool_p{i}", [pcs, PROJ], BF16, kind="Internal").ap()
        for i in range(NCHUNK_P)
    ]
    pool_full = nc.dram_tensor(
        "pool_full", [pool_rows, PROJ], BF16, kind="Internal", addr_space="Shared"
    ).ap()

    groups = [list(range(ncores))]

    with tile.TileContext(nc) as tc:
        with (
            tc.tile_pool(name="res", bufs=1) as res,
            tc.tile_pool(name="ld", bufs=6) as ld,
            tc.tile_pool(name="work", bufs=4) as work,
            tc.tile_pool(name="small", bufs=8) as small,
            tc.tile_pool(name="psh", bufs=3, space="PSUM") as psh,
            tc.tile_pool(name="psp", bufs=3, space="PSUM") as psp,
            tc.tile_pool(name="pst", bufs=2, space="PSUM") as pst,
        ):
            # ---- resident constants ----
            w1_sb = res.tile([P, HID // P, MID // P, P], BF16)
            nc.scalar.dma_start(out=w1_sb, in_=w1)
            w2_sb = res.tile([P, MID // P, PROJ], BF16)
            nc.scalar.dma_start(out=w2_sb, in_=w2)
            id_sb = res.tile([P, P], BF16)
            nc.scalar.dma_start(out=id_sb, in_=ident)
            if affine:
                eps_sb = res.tile([P, 1], F32)
                nc.vector.memset(eps_sb, EPS_LN)
            if use_b1:
                b1_sb = res.tile([P, MID // P], F32)
                nc.scalar.dma_start(out=b1_sb, in_=b1)
            if use_b2:
                b2_sb = res.tile([P, 1], F32)
                nc.scalar.dma_start(out=b2_sb, in_=b2)
            if use_gamma:
                gam_bc = res.tile([P, PROJ], F32)
                nc.gpsimd.dma_start(
                    out=gam_bc,
                    in_=bass.AP(tensor=gam.tensor, offset=0, ap=[[0, P], [1, PROJ]]),
                )
            if use_beta:
                bet_bc = res.tile([P, PROJ], F32)
                nc.gpsimd.dma_start(
                    out=bet_bc,
                    in_=bass.AP(tensor=bet.tensor, offset=0, ap=[[0, P], [1, PROJ]]),
                )

            aq = res.tile([P, nt, PROJ], BF16)
            pq = res.tile([P, nt, PROJ], BF16)
            NS = (K + 1) if gb == "exact" else (4 * K + 1)
            sims_all = res.tile([P, nt, NS], F32)
            if gb != "exact":
                pen_sb = res.tile([P, nt, 4 * K], BF16)
                nc.scalar.dma_start(out=pen_sb, in_=pen)

            # ---- phase 1: projection MLP (feature-major) ----
            def project_input(xT_dram, qdst, pool_parts):
                # pool_parts: list of (dram_ap, first_block, n_blocks)
                for blk in range(nblk):
                    r0 = blk * NB
                    xTb = ld.tile([P, HID // P, NB], BF16, tag="xTb")
                    nc.scalar.dma_start(
                        out=xTb,
                        in_=xT_dram[:, r0 : r0 + NB].rearrange(
                            "(c p) n -> p c n", p=P
                        ),
                    )
                    hT = work.tile([P, MID // P, NB], BF16, tag="hT")
                    for mc in range(MID // P):
                        ph = psh.tile([P, NB], F32, tag="ph")
                        for kc in range(HID // P):
                            nc.tensor.matmul(
                                ph,
                                lhsT=w1_sb[:, kc, mc, :],
                                rhs=xTb[:, kc, :],
                                start=(kc == 0),
                                stop=(kc == HID // P - 1),
                            )
                        # ReLU (+b1) PSUM -> SBUF bf16
                        nc.scalar.activation(
                            out=hT[:, mc, :],
                            in_=ph,
                            func=AF.Relu,
                            bias=b1_sb[:, mc : mc + 1] if use_b1 else 0.0,
                        )
                    # p.T [128, NB] fp32 in PSUM
                    pp = psp.tile([P, NB], F32, tag="pp")
                    for mc in range(MID // P):
                        nc.tensor.matmul(
                            pp,
                            lhsT=w2_sb[:, mc, :],
                            rhs=hT[:, mc, :],
                            start=(mc == 0),
                            stop=(mc == MID // P - 1),
                        )
                    # p.T -> SBUF bf16 (+b2)
                    pt = work.tile([P, NB], BF16, tag="pt")
                    if use_b2:
                        nc.scalar.activation(
                            out=pt, in_=pp, func=AF.Identity, bias=b2_sb
                        )
                    else:
                        nc.scalar.activation(out=pt, in_=pp, func=AF.Copy)
                    # PE transpose (xbar DMA-transpose stalls behind in-flight
                    # collectives): [128, NB] -> [128, NB/128, 128] in PSUM
                    NC4 = NB // P
                    p4 = pst.tile([P, NC4, P], BF16, tag="p4")
                    for c in range(NC4):
                        nc.tensor.transpose(
                            p4[:, c, :], pt[:, c * P : (c + 1) * P], id_sb
                        )
                    stats4 = small.tile([P, NC4, 6], F32, tag="stats4")
                    for c in range(NC4):
                        nc.vector.bn_stats(out=stats4[:, c, :], in_=p4[:, c, :])
                    mv4 = small.tile([P, NC4, 2], F32, tag="mv4")
                    for c in range(NC4):
                        nc.vector.bn_aggr(
                            out=mv4[:, c, :], in_=stats4[:, c : c + 1, :]
                        )
                    if not affine:
                        # LN + L2-normalize with gamma=1, beta=0 is exactly
                        # (p - mu) * rsqrt(PROJ * var): eps cancels.
                        sdv = small.tile([P, NC4], F32, tag="sdv")
                        nc.scalar.activation(
                            out=sdv,
                            in_=mv4[:, :, 1:2].rearrange("p c o -> p (c o)"),
                            func=AF.Sqrt,
                            scale=float(PROJ),
                        )
                        rs4 = small.tile([P, NC4], F32, tag="rs4")
                        nc.vector.reciprocal(out=rs4, in_=sdv)
                        for c in range(NC4):
                            t = blk * NC4 + c
                            nc.vector.tensor_scalar(
                                out=qdst[:, t, :],
                                in0=p4[:, c, :],
                                scalar1=mv4[:, c, 0:1],
                                scalar2=rs4[:, c : c + 1],
                                op0=OP.subtract,
                                op1=OP.mult,
                            )
                    else:
                        sd4 = small.tile([P, NC4], F32, tag="sd4")
                        nc.scalar.activation(
                            out=sd4,
                            in_=mv4[:, :, 1:2].rearrange("p c o -> p (c o)"),
                            func=AF.Sqrt,
                            bias=eps_sb,
                        )
                        rstd4 = small.tile([P, NC4], F32, tag="rstd4")
                        nc.vector.reciprocal(out=rstd4, in_=sd4)
                        ss4 = small.tile([P, NC4], F32, tag="ss4")
                        q0s = work.tile([P, NC4, PROJ], F32, tag="q0s")
                        for c in range(NC4):
                            q0 = q0s[:, c, :]
                            nc.vector.tensor_scalar(
                                out=q0,
                                in0=p4[:, c, :],
                                scalar1=mv4[:, c, 0:1],
                                scalar2=rstd4[:, c : c + 1],
                                op0=OP.subtract,
                                op1=OP.mult,
                            )
                            if use_gamma:
                                nc.vector.tensor_mul(out=q0, in0=q0, in1=gam_bc)
                            if use_beta:
                                nc.vector.tensor_add(out=q0, in0=q0, in1=bet_bc)
                            qsq = work.tile([P, PROJ], BF16, tag="qsq")
                            nc.vector.scalar_tensor_tensor(
                                out=qsq,
                                in0=q0,
                                scalar=1.0,
                                in1=q0,
                                op0=OP.mult,
                                op1=OP.mult,
                                accum_out=ss4[:, c : c + 1],
                            )
                        sd24 = small.tile([P, NC4], F32, tag="sd24")
                        nc.scalar.activation(out=sd24, in_=ss4, func=AF.Sqrt)
                        rno4 = small.tile([P, NC4], F32, tag="rno4")
                        nc.vector.reciprocal(out=rno4, in_=sd24)
                        for c in range(NC4):
                            t = blk * NC4 + c
                            nc.vector.tensor_scalar(
                                out=qdst[:, t, :],
                                in0=q0s[:, c, :],
                                scalar1=rno4[:, c : c + 1],
                                scalar2=None,
                                op0=OP.mult,
                            )
                    # pool shard write (sync queue: keep the scalar HWDGE ring
                    # free for the next input's xTb loads)
                    for part_ap, pb0, pnb in pool_parts:
                        if pb0 <= blk < pb0 + pnb:
                            pr0 = (blk - pb0) * NB
                            nc.sync.dma_start(
                                out=part_ap[pr0 : pr0 + NB, :].rearrange(
                                    "(c p) d -> p c d", p=P
                                ),
                                in_=qdst[
                                    :, blk * (NB // P) : (blk + 1) * (NB // P), :
                                ],
                            )

            # both inputs' pool shards are chunked; each chunk AllGathers as
            # soon as its blocks are written, hiding the serial CC chain
            # under the remaining projection work.
            bpa = nblk // NCHUNK_A
            project_input(
                xa, aq, [(pool_ac[i], i * bpa, bpa) for i in range(NCHUNK_A)]
            )
            for i in range(NCHUNK_A):
                nc.gpsimd.collective_compute(
                    kind="AllGather",
                    op=OP.bypass,
                    replica_groups=groups,
                    ins=[pool_ac[i]],
                    outs=[
                        pool_full[i * acs * ncores : (i + 1) * acs * ncores, :]
                    ],
                )
            bpc = nblk // NCHUNK_P
            base = bs * ncores
            project_input(
                xp, pq, [(pool_pc[i], i * bpc, bpc) for i in range(NCHUNK_P)]
            )
            for i in range(NCHUNK_P):
                nc.gpsimd.collective_compute(
                    kind="AllGather",
                    op=OP.bypass,
                    replica_groups=groups,
                    ins=[pool_pc[i]],
                    outs=[
                        pool_full[
                            base + i * pcs * ncores : base + (i + 1) * pcs * ncores,
                            :,
                        ]
                    ],
                )

            # pos sims — run during the positives AllGathers
            PSC = 8 if nt % 8 == 0 else 1
            for t0 in range(0, nt, PSC):
                prodp = work.tile([P, PSC, PROJ], BF16, tag="prodp")
                nc.vector.tensor_tensor(
                    out=prodp,
                    in0=aq[:, t0 : t0 + PSC, :],
                    in1=pq[:, t0 : t0 + PSC, :],
                    op=OP.mult,
                )
                nc.vector.tensor_reduce(
                    out=sims_all[:, t0 : t0 + PSC, NS - 1 : NS],
                    in_=prodp,
                    axis=mybir.AxisListType.X,
                    op=OP.add,
                )

            # ---- phase 2: gather negatives + sims ----
            if gb == "exact":
                # exact-row gather: one indirect DMA per tile with [128, K]
                # int32 row offsets — no super-row candidates, no penalties.
                IDXC = 8 if nt % 8 == 0 else 1
                for ch in range(nt // IDXC):
                    t0 = ch * IDXC
                    idxs = ld.tile([P, IDXC, K], I32, tag="idxs", bufs=3)
                    nc.scalar.dma_start(
                        out=idxs,
                        in_=nidx[t0 * P : (t0 + IDXC) * P, :].rearrange(
                            "(c p) k -> p c k", p=P
                        ),
                    )
                    for c in range(IDXC):
                        t = t0 + c
                        negs = work.tile([P, K, PROJ], BF16, tag="negs", bufs=6)
                        nc.gpsimd.indirect_dma_start(
                            out=negs,
                            out_offset=None,
                            in_=pool_full,
                            in_offset=bass.IndirectOffsetOnAxis(
                                ap=idxs[:, c, :], axis=0
                            ),
                        )
                        abase = aq[:, t, :]
                        abc = bass.AP(
                            tensor=abase.tensor,
                            offset=abase.offset,
                            ap=[list(abase.ap[0]), [0, K], list(abase.ap[-1])],
                        )
                        prodn = work.tile([P, K, PROJ], BF16, tag="prodn", bufs=4)
                        nc.vector.tensor_tensor(
                            out=prodn, in0=negs, in1=abc, op=OP.mult
                        )
                        nc.vector.tensor_reduce(
                            out=sims_all[:, t, 0:K],
                            in_=prodn,
                            axis=mybir.AxisListType.X,
                            op=OP.add,
                        )
                super_chunks = []
            else:
                IDXC = 8 if (nt % 8 == 0 and gch == 1) else gch
                super_chunks = list(range(nt // IDXC))
            pool_super = pool_full.rearrange("(a b) d -> a (b d)", b=4)
            for ch in super_chunks:
                t0 = ch * IDXC
                gi = ld.tile(
                    [P, IDXC // gch, gch * K * P // 16], I16, tag="gi", bufs=3
                )
                nc.scalar.dma_start(
                    out=gi, in_=gidx[t0 // gch : (t0 + IDXC) // gch, :, :].rearrange(
                        "c p s -> p c s"
                    )
                )
                for gg in range(IDXC // gch):
                    g = t0 // gch + gg
                    negs4 = work.tile(
                        [P, gch * K, 4 * PROJ], BF16, tag="negs4", bufs=4
                    )
                    nc.gpsimd.dma_gather(
                        out_ap=negs4,
                        in_ap=pool_super,
                        idxs_ap=gi[:, gg, :],
                        num_idxs=gch * K * P,
                        num_idxs_reg=gch * K * P,
                        elem_size=4 * PROJ,
                        queue_num=g % nsq,
                        single_packet=os.environ.get("BTCL_SP", "1") == "1",
                    )
                    for c in range(gch):
                        t = g * gch + c
                        abase = aq[:, t, :]
                        abc = bass.AP(
                            tensor=abase.tensor,
                            offset=abase.offset,
                            ap=[list(abase.ap[0]), [0, 4 * K], list(abase.ap[-1])],
                        )
                        prod4 = work.tile(
                            [P, 4 * K, PROJ], BF16, tag="prod4", bufs=4
                        )
                        nc.vector.tensor_tensor(
                            out=prod4,
                            in0=negs4[:, c * K : (c + 1) * K, :].rearrange(
                                "p k (s d) -> p (k s) d", d=PROJ
                            ),
                            in1=abc,
                            op=OP.mult,
                        )
                        # first fold on GpSimd — it only does gather desc-gen
                        # in phase 2, while DVE is the phase-2 bottleneck
                        fold = work.tile(
                            [P, 4 * K, PROJ // 2], BF16, tag="fold", bufs=2
                        )
                        nc.gpsimd.tensor_add(
                            out=fold,
                            in0=prod4[:, :, 0 : PROJ // 2],
                            in1=prod4[:, :, PROJ // 2 : PROJ],
                        )
                        fold2 = work.tile(
                            [P, 4 * K, PROJ // 4], BF16, tag="fold2", bufs=2
                        )
                        nc.vector.tensor_add(
                            out=fold2,
                            in0=fold[:, :, 0 : PROJ // 4],
                            in1=fold[:, :, PROJ // 4 : PROJ // 2],
                        )
                        nc.vector.tensor_reduce(
                            out=sims_all[:, t, 0 : 4 * K],
                            in_=fold2,
                            axis=mybir.AxisListType.X,
                            op=OP.add,
                        )

            # one batched penalty add for all tiles (super-row mode only)
            if gb != "exact":
                nc.vector.tensor_add(
                    out=sims_all[:, :, 0 : 4 * K],
                    in0=sims_all[:, :, 0 : 4 * K],
                    in1=pen_sb,
                )

            # ---- batched logsumexp over all staged sims ----
            e_all = work.tile([P, nt, NS], F32, tag="e_all", bufs=1)
            nc.scalar.activation(
                out=e_all, in_=sims_all, func=AF.Exp, scale=1.0 / TEMP
            )
            se = work.tile([P, nt], F32, tag="se", bufs=1)
            nc.vector.tensor_reduce(
                out=se, in_=e_all, axis=mybir.AxisListType.X, op=OP.add
            )
            lse = work.tile([P, nt], F32, tag="lse", bufs=1)
            nc.scalar.activation(out=lse, in_=se, func=AF.Ln)
            acc2 = res.tile([P, 2], F32)
            nc.vector.tensor_reduce(
                out=acc2[:, 0:1], in_=lse, axis=mybir.AxisListType.X, op=OP.add
            )
            nc.vector.tensor_reduce(
                out=acc2[:, 1:2],
                in_=sims_all[:, :, NS - 1 : NS].rearrange("p n o -> p (n o)"),
                axis=mybir.AxisListType.X,
                op=OP.add,
            )
            nc.scalar.dma_start(out=out_acc, in_=acc2)

    nc.finalize()
    return nc


_BUILD_CACHE: dict = {}
LAST_RESULTS = None


def _get_built(key):
    if key not in _BUILD_CACHE:
        bs, ncores, ub1, ub2, ug, ube, _gb = key
        _BUILD_CACHE[key] = build_loss_kernel(bs, ncores, ub1, ub2, ug, ube)
    return _BUILD_CACHE[key]


def _remap_pos_chunked(j: np.ndarray, B: int, bs: int, ncores: int) -> np.ndarray:
    """Map reference pool indices to the chunked-AllGather pool layout.

    With chunked AGs, shard row b = r*bs + l of chunk q = l // cs lands at
    half_base + q*cs*ncores + r*cs + (l % cs)."""
    half = (j >= B).astype(j.dtype)
    b = j - half * B
    r = b // bs
    l = b % bs
    acs = bs // NCHUNK_A
    pcs = bs // NCHUNK_P
    newa = (l // acs) * acs * ncores + r * acs + (l % acs)
    newp = B + (l // pcs) * pcs * ncores + r * pcs + (l % pcs)
    return np.where(half > 0, newp, newa).astype(j.dtype)


def _prep_gather_indices(ni: np.ndarray, gch: int):
    """ni: [bs, K] int32 ->
    (gidx [nt/gch, 128, gch*K*128//16] int16, pen [128, nt, 4K] bf16).

    gidx: per gather-group flat super-row index list (s = t_in_group*K*128 +
    k*128 + p), wrapped into 16 partitions and replicated to 128.
    pen[p, t, k*4+s] = 0 where s is the sub-row of neg_idx[t*128+p, k],
    else -8 (kills the candidate in the logsumexp)."""
    bf = ml_dtypes.bfloat16
    bs = ni.shape[0]
    nt = bs // P
    sup = (ni >> 2).astype(np.int16)  # super-row (1KB bf16) index, <= 32767
    sub = (ni & 3).astype(np.int64)
    ni3 = sup.reshape(nt, P, K)
    ng = nt // gch
    gidx = np.empty((ng, P, gch * K * P // 16), dtype=np.int16)
    for g in range(ng):
        flat = np.concatenate(
            [ni3[g * gch + c].T.ravel() for c in range(gch)]
        )  # s = c*K*128 + k*128 + p
        t16 = np.ascontiguousarray(
            flat.reshape(gch * K * P // 16, 16).T
        )  # [16, s16]
        gidx[g] = np.tile(t16, (P // 16, 1))
    pen = np.full((nt, P, K, 4), -8.0, dtype=np.float32)
    s3 = sub.reshape(nt, P, K)
    tt, pp2, kk = np.meshgrid(
        np.arange(nt), np.arange(P), np.arange(K), indexing="ij"
    )
    pen[tt, pp2, kk, s3] = 0.0
    penT = np.ascontiguousarray(
        pen.reshape(nt, P, 4 * K).transpose(1, 0, 2)
    ).astype(bf)
    return gidx, penT


def kernel(**inputs) -> np.ndarray:
    bf = ml_dtypes.bfloat16
    hs = np.asarray(inputs["hidden_states"], dtype=np.float32)
    ph = np.asarray(inputs["positive_hidden"], dtype=np.float32)
    ni = np.asarray(inputs["neg_idx"]).astype(np.int32)
    W1 = np.asarray(inputs["W1"], dtype=np.float32)
    b1 = np.asarray(inputs["b1"], dtype=np.float32)
    W2 = np.asarray(inputs["W2"], dtype=np.float32)
    b2 = np.asarray(inputs["b2"], dtype=np.float32)
    g = np.asarray(inputs["ln_gamma"], dtype=np.float32)
    be = np.asarray(inputs["ln_beta"], dtype=np.float32)

    B = hs.shape[0]
    ncores = N_CORES
    bs = B // ncores
    gch = int(os.environ.get("BTCL_GCH", str(GCH)))

    use_b1 = bool(np.any(b1))
    use_b2 = bool(np.any(b2))
    use_gamma = not bool(np.all(g == 1.0))
    use_beta = bool(np.any(be))
    gb = os.environ.get("BTCL_GATHER", "super")

    nc = _get_built((bs, ncores, use_b1, use_b2, use_gamma, use_beta, gb))

    w1d = np.ascontiguousarray(
        W1.reshape(HID // P, P, MID // P, P).transpose(1, 0, 2, 3).astype(bf)
    )
    w2d = np.ascontiguousarray(
        W2.reshape(MID // P, P, PROJ).transpose(1, 0, 2).astype(bf)
    )

    hsT = np.ascontiguousarray(hs.T.astype(bf))  # [HID, B] bf16
    phT = np.ascontiguousarray(ph.T.astype(bf))

    in_maps = []
    for c in range(ncores):
        m = {
            "xa": np.ascontiguousarray(hsT[:, c * bs : (c + 1) * bs]),
            "xp": np.ascontiguousarray(phT[:, c * bs : (c + 1) * bs]),
            "w1": w1d,
            "w2": w2d,
            "ident": np.eye(P, dtype=bf),
        }
        nic = _remap_pos_chunked(ni[c * bs : (c + 1) * bs], B, bs, ncores)
        if gb == "exact":
            m["nidx"] = np.ascontiguousarray(nic.astype(np.int32))
        else:
            gidx, pen = _prep_gather_indices(nic, gch)
            m["gidx"] = gidx
            m["pen"] = pen
        if use_b1:
            m["b1v"] = np.ascontiguousarray(
                b1.reshape(MID // P, P).T.astype(np.float32)
            )
        if use_b2:
            m["b2v"] = np.ascontiguousarray(b2.reshape(P, 1).astype(np.float32))
        if use_gamma:
            m["gam"] = np.ascontiguousarray(g.reshape(1, PROJ))
        if use_beta:
            m["bet"] = np.ascontiguousarray(be.reshape(1, PROJ))
        in_maps.append(m)

    trace = bool(int(os.environ.get("BTCL_TRACE", "0")))
    res = run_bass_kernel_spmd(
        nc, in_maps, core_ids=list(range(ncores)), trace=trace
    )
    global LAST_RESULTS
    LAST_RESULTS = res
    acc = np.zeros((P, 2), dtype=np.float64)
    for r in res.results:
        acc += r["out_acc"].astype(np.float64)
    # acc[:, 0] = sum(logsumexp(sims/T)); acc[:, 1] = sum(pos_sim) (unscaled)
    loss = (acc[:, 0].sum() - acc[:, 1].sum() / TEMP) / float(B)
    return np.float32(loss)



# revision 44
# speedup vs baseline: 3.6085x; 3.6085x over previous
"""Trainium2 Bass kernel for BatchTemporalContrastiveLoss.

Computation (see reference):
  project(x) = l2norm(layernorm(relu(x @ W1 + b1) @ W2 + b2) * g + beta)
  anchors    = project(hidden_states)     [B, 128]
  positives  = project(positive_hidden)   [B, 128]
  pool       = concat([anchors, positives])          [2B, 128]
  negs       = pool[neg_idx]                         [B, K, 128]
  loss       = mean(-pos_sim + logsumexp([pos_sim, neg_sims]))

Strategy: pure data-parallel over B across 8 NeuronCores.
- Host pre-transposes activations to bf16 feature-major shards; the MLP runs
  feature-major (no on-chip input transposes); one wide DMA-xbar transpose
  per 512-row block flips projections to row-major for the normalize stage.
- gamma==1/beta==0 fast path: LN + L2-normalize collapses to
  (p - mu) * rsqrt(128*var)  (the LN eps cancels exactly), halving the
  normalize DVE work.
- bf16 pool shards to DRAM; the anchors AllGather overlaps the positives
  projection; the positives AllGather is split into chunks so most of it
  hides under the projection as well.
- Negatives fetched with dma_gather in merged 4-tile calls (4096 indices
  per instruction) to amortize per-instruction drain serialization:
  indices >> 2 select 1KB super-rows (4 bf16 rows, fits int16); a
  host-prepared penalty mask (-8 on wrong sub-rows) kills bad candidates
  inside the final logsumexp.
- Similarities: DVE mul + direct tensor_reduce; one batched logsumexp at
  the end. Host sums 8 per-core [128, 2] accumulators.
"""

import os
import sys

import numpy as np

for _p in ("/opt/trn_rl_repo", "/root/.axon_site/_ro/trn_rl_repo"):
    if os.path.isdir(_p) and _p not in sys.path:
        sys.path.append(_p)

import ml_dtypes  # noqa: E402

import concourse.bacc as bacc  # noqa: E402
import concourse.bass as bass  # noqa: E402
import concourse.tile as tile  # noqa: E402
from concourse import mybir  # noqa: E402
from concourse.bass_utils import run_bass_kernel_spmd  # noqa: E402

F32 = mybir.dt.float32
BF16 = mybir.dt.bfloat16
FP8 = mybir.dt.float8e4
I16 = mybir.dt.int16
I32 = mybir.dt.int32
PM2 = mybir.MatmulPerfMode.DoubleRow
AF = mybir.ActivationFunctionType
OP = mybir.AluOpType

HID = 512
MID = 256
PROJ = 128
K = 8
TEMP = 0.07
EPS_LN = 1e-5
P = 128
NB = 512  # rows per phase-1 block

N_CORES = 8
B_FULL = 65536
BS = B_FULL // N_CORES  # rows per core per input (8192)

NCHUNK_A = 1  # anchors AllGather chunks (chunking loses AG bandwidth)
NCHUNK_P = 1  # positives AllGather chunks
GCH = 1  # tiles merged per dma_gather call (>2 overflows the SWDGE desc ring)


def build_loss_kernel(
    bs: int = BS,
    ncores: int = N_CORES,
    use_b1: bool = False,
    use_b2: bool = False,
    use_gamma: bool = False,
    use_beta: bool = False,
):
    nt = bs // P  # 128-row tiles per input
    nblk = bs // NB  # 512-row blocks per input
    assert bs % NB == 0
    pool_rows = 2 * bs * ncores
    affine = use_gamma or use_beta

    nsq = int(os.environ.get("BTCL_NSQ", "4"))
    gch = int(os.environ.get("BTCL_GCH", str(GCH)))
    gb = os.environ.get("BTCL_GATHER", "super")  # "exact" | "super"
    fp8mm = os.environ.get("BTCL_FP8MM", "0") == "1"
    mdt = FP8 if fp8mm else BF16  # matmul operand dtype
    nc = bacc.Bacc(trn_type="TRN2", num_devices=ncores, num_swdge_queues=nsq)

    # Per-core external inputs. xa/xp are TRANSPOSED shards [HID, bs].
    xa = nc.dram_tensor("xa", [HID, bs], mdt, kind="ExternalInput").ap()
    xp = nc.dram_tensor("xp", [HID, bs], mdt, kind="ExternalInput").ap()
    # w1 blocks: w1[p, kc, mc, j] = W1[kc*128+p, mc*128+j]
    w1 = nc.dram_tensor(
        "w1", [P, HID // P, MID // P, P], mdt, kind="ExternalInput"
    ).ap()
    # w2 blocks: w2[p, kc, j] = W2[kc*128+p, j]
    w2 = nc.dram_tensor("w2", [P, MID // P, PROJ], mdt, kind="ExternalInput").ap()
    if gb == "exact":
        nidx = nc.dram_tensor("nidx", [bs, K], I32, kind="ExternalInput").ap()
        gidx = pen = None
    else:
        # merged gather indices: [nt/gch, 128, gch*K*128/16]
        gidx = nc.dram_tensor(
            "gidx", [nt // gch, P, gch * K * P // 16], I16, kind="ExternalInput"
        ).ap()
        # penalties pre-transposed on host to [128, nt, 4K]
        pen = nc.dram_tensor("pen", [P, nt, 4 * K], BF16, kind="ExternalInput").ap()
        nidx = None
    ident = nc.dram_tensor("ident", [P, P], BF16, kind="ExternalInput").ap()
    b1 = (
        nc.dram_tensor("b1v", [P, MID // P], F32, kind="ExternalInput").ap()
        if use_b1
        else None
    )
    b2 = (
        nc.dram_tensor("b2v", [P, 1], F32, kind="ExternalInput").ap()
        if use_b2
        else None
    )
    gam = (
        nc.dram_tensor("gam", [1, PROJ], F32, kind="ExternalInput").ap()
        if use_gamma
        else None
    )
    bet = (
        nc.dram_tensor("bet", [1, PROJ], F32, kind="ExternalInput").ap()
        if use_beta
        else None
    )

    out_acc = nc.dram_tensor("out_acc", [P, 2], F32, kind="ExternalOutput").ap()

    acs = bs // NCHUNK_A  # anchor-shard chunk rows
    pool_ac = [
        nc.dram_tensor(f"pool_a{i}", [acs, PROJ], BF16, kind="Internal").ap()
        for i in range(NCHUNK_A)
    ]
    pcs = bs // NCHUNK_P  # positive-shard chunk rows
    pool_pc = [
        nc.dram_tensor(f"pool_p{i}", [pcs, PROJ], BF16, kind="Internal").ap()
        for i in range(NCHUNK_P)
    ]
    pool_full = nc.dram_tensor(
        "pool_full", [pool_rows, PROJ], BF16, kind="Internal", addr_space="Shared"
    ).ap()

    groups = [list(range(ncores))]

    with tile.TileContext(nc) as tc:
        with (
            tc.tile_pool(name="res", bufs=1) as res,
            tc.tile_pool(name="ld", bufs=6) as ld,
            tc.tile_pool(name="work", bufs=4) as work,
            tc.tile_pool(name="small", bufs=8) as small,
            tc.tile_pool(name="psh", bufs=3, space="PSUM") as psh,
            tc.tile_pool(name="psp", bufs=3, space="PSUM") as psp,
            tc.tile_pool(name="pst", bufs=2, space="PSUM") as pst,
        ):
            # ---- resident constants ----
            w1_sb = res.tile([P, HID // P, MID // P, P], mdt)
            nc.scalar.dma_start(out=w1_sb, in_=w1)
            w2_sb = res.tile([P, MID // P, PROJ], mdt)
            nc.scalar.dma_start(out=w2_sb, in_=w2)
            id_sb = res.tile([P, P], BF16)
            nc.scalar.dma_start(out=id_sb, in_=ident)
            if affine:
                eps_sb = res.tile([P, 1], F32)
                nc.vector.memset(eps_sb, EPS_LN)
            if use_b1:
                b1_sb = res.tile([P, MID // P], F32)
                nc.scalar.dma_start(out=b1_sb, in_=b1)
            if use_b2:
                b2_sb = res.tile([P, 1], F32)
                nc.scalar.dma_start(out=b2_sb, in_=b2)
            if use_gamma:
                gam_bc = res.tile([P, PROJ], F32)
                nc.gpsimd.dma_start(
                    out=gam_bc,
                    in_=bass.AP(tensor=gam.tensor, offset=0, ap=[[0, P], [1, PROJ]]),
                )
            if use_beta:
                bet_bc = res.tile([P, PROJ], F32)
                nc.gpsimd.dma_start(
                    out=bet_bc,
                    in_=bass.AP(tensor=bet.tensor, offset=0, ap=[[0, P], [1, PROJ]]),
                )

            aq = res.tile([P, nt, PROJ], BF16)
            pq = res.tile([P, nt, PROJ], BF16)
            NS = (K + 1) if gb == "exact" else (4 * K + 1)
            sims_all = res.tile([P, nt, NS], F32)
            if gb != "exact":
                pen_sb = res.tile([P, nt, 4 * K], BF16)
                nc.scalar.dma_start(out=pen_sb, in_=pen)

            # ---- phase 1: projection MLP (feature-major) ----
            def project_input(xT_dram, qdst, pool_parts):
                # pool_parts: list of (dram_ap, first_block, n_blocks)
                for blk in range(nblk):
                    r0 = blk * NB
                    xTb = ld.tile([P, HID // P, NB], mdt, tag="xTb")
                    nc.scalar.dma_start(
                        out=xTb,
                        in_=xT_dram[:, r0 : r0 + NB].rearrange(
                            "(c p) n -> p c n", p=P
                        ),
                    )
                    hT = work.tile([P, MID // P, NB], mdt, tag="hT")
                    for mc in range(MID // P):
                        ph = psh.tile([P, NB], F32, tag="ph")
                        if fp8mm:
                            # fp8 DoubleRow: two k-tiles per call, 2x PE rate
                            for k2 in range(HID // P // 2):
                                nc.tensor.matmul(
                                    ph,
                                    lhsT=w1_sb[:, 2 * k2 : 2 * k2 + 2, mc, :],
                                    rhs=xTb[:, 2 * k2 : 2 * k2 + 2, :],
                                    start=(k2 == 0),
                                    stop=(k2 == HID // P // 2 - 1),
                                    perf_mode=PM2,
                                )
                        else:
                            for kc in range(HID // P):
                                nc.tensor.matmul(
                                    ph,
                                    lhsT=w1_sb[:, kc, mc, :],
                                    rhs=xTb[:, kc, :],
                                    start=(kc == 0),
                                    stop=(kc == HID // P - 1),
                                )
                        # ReLU (+b1) PSUM -> SBUF
                        nc.scalar.activation(
                            out=hT[:, mc, :],
                            in_=ph,
                            func=AF.Relu,
                            bias=b1_sb[:, mc : mc + 1] if use_b1 else 0.0,
                        )
                    # p.T [128, NB] fp32 in PSUM
                    pp = psp.tile([P, NB], F32, tag="pp")
                    if fp8mm:
                        nc.tensor.matmul(
                            pp,
                            lhsT=w2_sb[:, :, :],
                            rhs=hT[:, :, :],
                            start=True,
                            stop=True,
                            perf_mode=PM2,
                        )
                    else:
                        for mc in range(MID // P):
                            nc.tensor.matmul(
                                pp,
                                lhsT=w2_sb[:, mc, :],
                                rhs=hT[:, mc, :],
                                start=(mc == 0),
                                stop=(mc == MID // P - 1),
                            )
                    # p.T -> SBUF bf16 (+b2)
                    pt = work.tile([P, NB], BF16, tag="pt")
                    if use_b2:
                        nc.scalar.activation(
                            out=pt, in_=pp, func=AF.Identity, bias=b2_sb
                        )
                    else:
                        nc.scalar.activation(out=pt, in_=pp, func=AF.Copy)
                    # PE transpose (xbar DMA-transpose stalls behind in-flight
                    # collectives): [128, NB] -> [128, NB/128, 128] in PSUM
                    NC4 = NB // P
                    p4 = pst.tile([P, NC4, P], BF16, tag="p4")
                    for c in range(NC4):
                        nc.tensor.transpose(
                            p4[:, c, :], pt[:, c * P : (c + 1) * P], id_sb
                        )
                    stats4 = small.tile([P, NC4, 6], F32, tag="stats4")
                    for c in range(NC4):
                        nc.vector.bn_stats(out=stats4[:, c, :], in_=p4[:, c, :])
                    mv4 = small.tile([P, NC4, 2], F32, tag="mv4")
                    for c in range(NC4):
                        nc.vector.bn_aggr(
                            out=mv4[:, c, :], in_=stats4[:, c : c + 1, :]
                        )
                    if not affine:
                        # LN + L2-normalize with gamma=1, beta=0 is exactly
                        # (p - mu) * rsqrt(PROJ * var): eps cancels.
                        sdv = small.tile([P, NC4], F32, tag="sdv")
                        nc.scalar.activation(
                            out=sdv,
                            in_=mv4[:, :, 1:2].rearrange("p c o -> p (c o)"),
                            func=AF.Sqrt,
                            scale=float(PROJ),
                        )
                        rs4 = small.tile([P, NC4], F32, tag="rs4")
                        nc.vector.reciprocal(out=rs4, in_=sdv)
                        for c in range(NC4):
                            t = blk * NC4 + c
                            nc.vector.tensor_scalar(
                                out=qdst[:, t, :],
                                in0=p4[:, c, :],
                                scalar1=mv4[:, c, 0:1],
                                scalar2=rs4[:, c : c + 1],
                                op0=OP.subtract,
                                op1=OP.mult,
                            )
                    else:
                        sd4 = small.tile([P, NC4], F32, tag="sd4")
                        nc.scalar.activation(
                            out=sd4,
                            in_=mv4[:, :, 1:2].rearrange("p c o -> p (c o)"),
                            func=AF.Sqrt,
                            bias=eps_sb,
                        )
                        rstd4 = small.tile([P, NC4], F32, tag="rstd4")
                        nc.vector.reciprocal(out=rstd4, in_=sd4)
                        ss4 = small.tile([P, NC4], F32, tag="ss4")
                        q0s = work.tile([P, NC4, PROJ], F32, tag="q0s")
                        for c in range(NC4):
                            q0 = q0s[:, c, :]
                            nc.vector.tensor_scalar(
                                out=q0,
                                in0=p4[:, c, :],
                                scalar1=mv4[:, c, 0:1],
                                scalar2=rstd4[:, c : c + 1],
                                op0=OP.subtract,
                                op1=OP.mult,
                            )
                            if use_gamma:
                                nc.vector.tensor_mul(out=q0, in0=q0, in1=gam_bc)
                            if use_beta:
                                nc.vector.tensor_add(out=q0, in0=q0, in1=bet_bc)
                            qsq = work.tile([P, PROJ], BF16, tag="qsq")
                            nc.vector.scalar_tensor_tensor(
                                out=qsq,
                                in0=q0,
                                scalar=1.0,
                                in1=q0,
                                op0=OP.mult,
                                op1=OP.mult,
                                accum_out=ss4[:, c : c + 1],
                            )
                        sd24 = small.tile([P, NC4], F32, tag="sd24")
                        nc.scalar.activation(out=sd24, in_=ss4, func=AF.Sqrt)
                        rno4 = small.tile([P, NC4], F32, tag="rno4")
                        nc.vector.reciprocal(out=rno4, in_=sd24)
                        for c in range(NC4):
                            t = blk * NC4 + c
                            nc.vector.tensor_scalar(
                                out=qdst[:, t, :],
                                in0=q0s[:, c, :],
                                scalar1=rno4[:, c : c + 1],
                                scalar2=None,
                                op0=OP.mult,
                            )
                    # pool shard write (sync queue: keep the scalar HWDGE ring
                    # free for the next input's xTb loads)
                    for part_ap, pb0, pnb in pool_parts:
                        if pb0 <= blk < pb0 + pnb:
                            pr0 = (blk - pb0) * NB
                            nc.sync.dma_start(
                                out=part_ap[pr0 : pr0 + NB, :].rearrange(
                                    "(c p) d -> p c d", p=P
                                ),
                                in_=qdst[
                                    :, blk * (NB // P) : (blk + 1) * (NB // P), :
                                ],
                            )

            # both inputs' pool shards are chunked; each chunk AllGathers as
            # soon as its blocks are written, hiding the serial CC chain
            # under the remaining projection work.
            bpa = nblk // NCHUNK_A
            project_input(
                xa, aq, [(pool_ac[i], i * bpa, bpa) for i in range(NCHUNK_A)]
            )
            for i in range(NCHUNK_A):
                nc.gpsimd.collective_compute(
                    kind="AllGather",
                    op=OP.bypass,
                    replica_groups=groups,
                    ins=[pool_ac[i]],
                    outs=[
                        pool_full[i * acs * ncores : (i + 1) * acs * ncores, :]
                    ],
                )
            bpc = nblk // NCHUNK_P
            base = bs * ncores
            project_input(
                xp, pq, [(pool_pc[i], i * bpc, bpc) for i in range(NCHUNK_P)]
            )
            for i in range(NCHUNK_P):
                nc.gpsimd.collective_compute(
                    kind="AllGather",
                    op=OP.bypass,
                    replica_groups=groups,
                    ins=[pool_pc[i]],
                    outs=[
                        pool_full[
                            base + i * pcs * ncores : base + (i + 1) * pcs * ncores,
                            :,
                        ]
                    ],
                )

            # pos sims — run during the positives AllGathers
            PSC = 8 if nt % 8 == 0 else 1
            for t0 in range(0, nt, PSC):
                prodp = work.tile([P, PSC, PROJ], BF16, tag="prodp")
                nc.vector.tensor_tensor(
                    out=prodp,
                    in0=aq[:, t0 : t0 + PSC, :],
                    in1=pq[:, t0 : t0 + PSC, :],
                    op=OP.mult,
                )
                nc.vector.tensor_reduce(
                    out=sims_all[:, t0 : t0 + PSC, NS - 1 : NS],
                    in_=prodp,
                    axis=mybir.AxisListType.X,
                    op=OP.add,
                )

            # ---- phase 2: gather negatives + sims ----
            if gb == "exact":
                # exact-row gather: one indirect DMA per tile with [128, K]
                # int32 row offsets — no super-row candidates, no penalties.
                IDXC = 8 if nt % 8 == 0 else 1
                for ch in range(nt // IDXC):
                    t0 = ch * IDXC
                    idxs = ld.tile([P, IDXC, K], I32, tag="idxs", bufs=3)
                    nc.scalar.dma_start(
                        out=idxs,
                        in_=nidx[t0 * P : (t0 + IDXC) * P, :].rearrange(
                            "(c p) k -> p c k", p=P
                        ),
                    )
                    for c in range(IDXC):
                        t = t0 + c
                        negs = work.tile([P, K, PROJ], BF16, tag="negs", bufs=6)
                        nc.gpsimd.indirect_dma_start(
                            out=negs,
                            out_offset=None,
                            in_=pool_full,
                            in_offset=bass.IndirectOffsetOnAxis(
                                ap=idxs[:, c, :], axis=0
                            ),
                        )
                        abase = aq[:, t, :]
                        abc = bass.AP(
                            tensor=abase.tensor,
                            offset=abase.offset,
                            ap=[list(abase.ap[0]), [0, K], list(abase.ap[-1])],
                        )
                        prodn = work.tile([P, K, PROJ], BF16, tag="prodn", bufs=4)
                        nc.vector.tensor_tensor(
                            out=prodn, in0=negs, in1=abc, op=OP.mult
                        )
                        nc.vector.tensor_reduce(
                            out=sims_all[:, t, 0:K],
                            in_=prodn,
                            axis=mybir.AxisListType.X,
                            op=OP.add,
                        )
                super_chunks = []
            else:
                IDXC = 8 if (nt % 8 == 0 and gch == 1) else gch
                super_chunks = list(range(nt // IDXC))
            pool_super = pool_full.rearrange("(a b) d -> a (b d)", b=4)
            for ch in super_chunks:
                t0 = ch * IDXC
                gi = ld.tile(
                    [P, IDXC // gch, gch * K * P // 16], I16, tag="gi", bufs=3
                )
                nc.scalar.dma_start(
                    out=gi, in_=gidx[t0 // gch : (t0 + IDXC) // gch, :, :].rearrange(
                        "c p s -> p c s"
                    )
                )
                for gg in range(IDXC // gch):
                    g = t0 // gch + gg
                    negs4 = work.tile(
                        [P, gch * K, 4 * PROJ], BF16, tag="negs4", bufs=4
                    )
                    nc.gpsimd.dma_gather(
                        out_ap=negs4,
                        in_ap=pool_super,
                        idxs_ap=gi[:, gg, :],
                        num_idxs=gch * K * P,
                        num_idxs_reg=gch * K * P,
                        elem_size=4 * PROJ,
                        queue_num=g % nsq,
                        single_packet=os.environ.get("BTCL_SP", "1") == "1",
                    )
                    for c in range(gch):
                        t = g * gch + c
                        abase = aq[:, t, :]
                        abc = bass.AP(
                            tensor=abase.tensor,
                            offset=abase.offset,
                            ap=[list(abase.ap[0]), [0, 4 * K], list(abase.ap[-1])],
                        )
                        prod4 = work.tile(
                            [P, 4 * K, PROJ], BF16, tag="prod4", bufs=4
                        )
                        nc.vector.tensor_tensor(
                            out=prod4,
                            in0=negs4[:, c * K : (c + 1) * K, :].rearrange(
                                "p k (s d) -> p (k s) d", d=PROJ
                            ),
                            in1=abc,
                            op=OP.mult,
                        )
                        fold = work.tile(
                            [P, 4 * K, PROJ // 2], BF16, tag="fold", bufs=2
                        )
                        nc.vector.tensor_add(
                            out=fold,
                            in0=prod4[:, :, 0 : PROJ // 2],
                            in1=prod4[:, :, PROJ // 2 : PROJ],
                        )
                        fold2 = work.tile(
                            [P, 4 * K, PROJ // 4], BF16, tag="fold2", bufs=2
                        )
                        nc.vector.tensor_add(
                            out=fold2,
                            in0=fold[:, :, 0 : PROJ // 4],
                            in1=fold[:, :, PROJ // 4 : PROJ // 2],
                        )
                        nc.vector.tensor_reduce(
                            out=sims_all[:, t, 0 : 4 * K],
                            in_=fold2,
                            axis=mybir.AxisListType.X,
                            op=OP.add,
                        )

            # one batched penalty add for all tiles (super-row mode only)
            if gb != "exact":
                nc.vector.tensor_add(
                    out=sims_all[:, :, 0 : 4 * K],
                    in0=sims_all[:, :, 0 : 4 * K],
                    in1=pen_sb,
                )

            # ---- batched logsumexp over all staged sims ----
            e_all = work.tile([P, nt, NS], F32, tag="e_all", bufs=1)
            nc.scalar.activation(
                out=e_all, in_=sims_all, func=AF.Exp, scale=1.0 / TEMP
            )
            se = work.tile([P, nt], F32, tag="se", bufs=1)
            nc.vector.tensor_reduce(
                out=se, in_=e_all, axis=mybir.AxisListType.X, op=OP.add
            )
            lse = work.tile([P, nt], F32, tag="lse", bufs=1)
            nc.scalar.activation(out=lse, in_=se, func=AF.Ln)
            acc2 = res.tile([P, 2], F32)
            nc.vector.tensor_reduce(
                out=acc2[:, 0:1], in_=lse, axis=mybir.AxisListType.X, op=OP.add
            )
            nc.vector.tensor_reduce(
                out=acc2[:, 1:2],
                in_=sims_all[:, :, NS - 1 : NS].rearrange("p n o -> p (n o)"),
                axis=mybir.AxisListType.X,
                op=OP.add,
            )
            nc.scalar.dma_start(out=out_acc, in_=acc2)

    nc.finalize()
    return nc


_BUILD_CACHE: dict = {}
LAST_RESULTS = None


def _get_built(key):
    if key not in _BUILD_CACHE:
        bs, ncores, ub1, ub2, ug, ube = key[:6]
        _BUILD_CACHE[key] = build_loss_kernel(bs, ncores, ub1, ub2, ug, ube)
    return _BUILD_CACHE[key]


def _remap_pos_chunked(j: np.ndarray, B: int, bs: int, ncores: int) -> np.ndarray:
    """Map reference pool indices to the chunked-AllGather pool layout.

    With chunked AGs, shard row b = r*bs + l of chunk q = l // cs lands at
    half_base + q*cs*ncores + r*cs + (l % cs)."""
    half = (j >= B).astype(j.dtype)
    b = j - half * B
    r = b // bs
    l = b % bs
    acs = bs // NCHUNK_A
    pcs = bs // NCHUNK_P
    newa = (l // acs) * acs * ncores + r * acs + (l % acs)
    newp = B + (l // pcs) * pcs * ncores + r * pcs + (l % pcs)
    return np.where(half > 0, newp, newa).astype(j.dtype)


def _prep_gather_indices(ni: np.ndarray, gch: int):
    """ni: [bs, K] int32 ->
    (gidx [nt/gch, 128, gch*K*128//16] int16, pen [128, nt, 4K] bf16).

    gidx: per gather-group flat super-row index list (s = t_in_group*K*128 +
    k*128 + p), wrapped into 16 partitions and replicated to 128.
    pen[p, t, k*4+s] = 0 where s is the sub-row of neg_idx[t*128+p, k],
    else -8 (kills the candidate in the logsumexp)."""
    bf = ml_dtypes.bfloat16
    bs = ni.shape[0]
    nt = bs // P
    sup = (ni >> 2).astype(np.int16)  # super-row (1KB bf16) index, <= 32767
    sub = (ni & 3).astype(np.int64)
    ni3 = sup.reshape(nt, P, K)
    ng = nt // gch
    gidx = np.empty((ng, P, gch * K * P // 16), dtype=np.int16)
    for g in range(ng):
        flat = np.concatenate(
            [ni3[g * gch + c].T.ravel() for c in range(gch)]
        )  # s = c*K*128 + k*128 + p
        t16 = np.ascontiguousarray(
            flat.reshape(gch * K * P // 16, 16).T
        )  # [16, s16]
        gidx[g] = np.tile(t16, (P // 16, 1))
    pen = np.full((nt, P, K, 4), -8.0, dtype=np.float32)
    s3 = sub.reshape(nt, P, K)
    tt, pp2, kk = np.meshgrid(
        np.arange(nt), np.arange(P), np.arange(K), indexing="ij"
    )
    pen[tt, pp2, kk, s3] = 0.0
    penT = np.ascontiguousarray(
        pen.reshape(nt, P, 4 * K).transpose(1, 0, 2)
    ).astype(bf)
    return gidx, penT


def kernel(**inputs) -> np.ndarray:
    bf = ml_dtypes.bfloat16
    hs = np.asarray(inputs["hidden_states"], dtype=np.float32)
    ph = np.asarray(inputs["positive_hidden"], dtype=np.float32)
    ni = np.asarray(inputs["neg_idx"]).astype(np.int32)
    W1 = np.asarray(inputs["W1"], dtype=np.float32)
    b1 = np.asarray(inputs["b1"], dtype=np.float32)
    W2 = np.asarray(inputs["W2"], dtype=np.float32)
    b2 = np.asarray(inputs["b2"], dtype=np.float32)
    g = np.asarray(inputs["ln_gamma"], dtype=np.float32)
    be = np.asarray(inputs["ln_beta"], dtype=np.float32)

    B = hs.shape[0]
    ncores = N_CORES
    bs = B // ncores
    gch = int(os.environ.get("BTCL_GCH", str(GCH)))

    use_b1 = bool(np.any(b1))
    use_b2 = bool(np.any(b2))
    use_gamma = not bool(np.all(g == 1.0))
    use_beta = bool(np.any(be))
    gb = os.environ.get("BTCL_GATHER", "super")
    fp8key = os.environ.get("BTCL_FP8MM", "0")

    nc = _get_built((bs, ncores, use_b1, use_b2, use_gamma, use_beta, gb, fp8key))

    fp8mm = os.environ.get("BTCL_FP8MM", "0") == "1"
    mdt_np = ml_dtypes.float8_e4m3 if fp8mm else bf
    w1d = np.ascontiguousarray(
        W1.reshape(HID // P, P, MID // P, P).transpose(1, 0, 2, 3).astype(mdt_np)
    )
    w2d = np.ascontiguousarray(
        W2.reshape(MID // P, P, PROJ).transpose(1, 0, 2).astype(mdt_np)
    )

    hsT = np.ascontiguousarray(hs.T.astype(mdt_np))  # [HID, B]
    phT = np.ascontiguousarray(ph.T.astype(mdt_np))

    in_maps = []
    for c in range(ncores):
        m = {
            "xa": np.ascontiguousarray(hsT[:, c * bs : (c + 1) * bs]),
            "xp": np.ascontiguousarray(phT[:, c * bs : (c + 1) * bs]),
            "w1": w1d,
            "w2": w2d,
            "ident": np.eye(P, dtype=bf),
        }
        nic = _remap_pos_chunked(ni[c * bs : (c + 1) * bs], B, bs, ncores)
        if gb == "exact":
            m["nidx"] = np.ascontiguousarray(nic.astype(np.int32))
        else:
            gidx, pen = _prep_gather_indices(nic, gch)
            m["gidx"] = gidx
            m["pen"] = pen
        if use_b1:
            m["b1v"] = np.ascontiguousarray(
                b1.reshape(MID // P, P).T.astype(np.float32)
            )
        if use_b2:
            m["b2v"] = np.ascontiguousarray(b2.reshape(P, 1).astype(np.float32))
        if use_gamma:
            m["gam"] = np.ascontiguousarray(g.reshape(1, PROJ))
        if use_beta:
            m["bet"] = np.ascontiguousarray(be.reshape(1, PROJ))
        in_maps.append(m)

    trace = bool(int(os.environ.get("BTCL_TRACE", "0")))
    res = run_bass_kernel_spmd(
        nc, in_maps, core_ids=list(range(ncores)), trace=trace
    )
    global LAST_RESULTS
    LAST_RESULTS = res
    acc = np.zeros((P, 2), dtype=np.float64)
    for r in res.results:
        acc += r["out_acc"].astype(np.float64)
    # acc[:, 0] = sum(logsumexp(sims/T)); acc[:, 1] = sum(pos_sim) (unscaled)
    loss = (acc[:, 0].sum() - acc[:, 1].sum() / TEMP) / float(B)
    return np.float32(loss)

